# revision 14
# baseline (speedup 1.0000x reference)
"""AVSNet adaptive voxel sampling kernel for Trainium2 (8 NeuronCores).

Pipeline:
  host (tiny, control-heavy):  PI-control voxel-size adaptation, voxel hashing,
      sort-based centroid computation  (exact replica of the reference, run on
      jax-CPU so float semantics match bit-for-bit).
  device (heavy):  for each of the first 4096 voxel-centroid columns of each
      batch, argmin over all 16384 points of squared distance.  Computed as
      argmax over points of t' = 2<c_j, x_i> - |x_i|^2  (the +|c_j|^2 term is
      constant per column and cannot change the argmin ordering beyond ulp-level
      coincidences).  One K=4 fp32 matmul per tile produces t' directly in PSUM
      (weights [2*c; 1], moving operand [x; -|x|^2]); Max8 + MaxIndex extract
      per-chunk top-8 values and first-occurrence indices.
  host (combine): exact re-evaluation of the ~64 candidate points per column
      with the reference's own formula, first-occurrence tie-breaking.

Sharding: core c -> batch c//4, centroid columns (c%4)*1024 ... +1024.
"""

import os
import sys

import numpy as np

if "/opt/trn_rl_repo" not in sys.path:
    sys.path.insert(0, "/opt/trn_rl_repo")

# ---------------------------------------------------------------- constants
V0 = 0.02
KP = 0.5
KI = 0.1
MAX_ITER = 10
B, N = 2, 16384
NPOINT = 4096

JSLICE = 1024            # centroid columns per core
NBLK = JSLICE // 128     # partition blocks per core
CS = 2048                # point-chunk size per Max8/MaxIndex call
NCH = N // CS            # chunks per block
MM_N = 512               # moving-operand width per fp32 matmul

_CACHE = {}


# ------------------------------------------------------------ host reference
# Verbatim replicas of the reference's control-heavy parts, executed on the
# jax CPU backend so rounding matches the oracle exactly.

def _host_parts(xyz_np, npoint):
    import jax
    import jax.numpy as jnp

    cpu = jax.devices("cpu")[0]
    with jax.default_device(cpu):
        xyz = jnp.asarray(xyz_np)

        def _keys(xyz, voxel_size):
            c = jnp.floor(xyz / voxel_size).astype(jnp.int32)
            return c[..., 0] * 1_000_000 + c[..., 1] * 1_000 + c[..., 2]

        def _count_unique(k):
            sk = jnp.sort(k)
            return 1 + jnp.sum((sk[1:] != sk[:-1]).astype(jnp.int32))

        def _centroids(xyz_b, k):
            n = k.shape[0]
            order = jnp.argsort(k)
            sk = k[order]
            is_new = jnp.concatenate([jnp.ones((1,), bool), sk[1:] != sk[:-1]])
            seg = jnp.cumsum(is_new.astype(jnp.int32)) - 1
            inv = jnp.zeros(n, jnp.int32).at[order].set(seg)
            sums = jax.ops.segment_sum(xyz_b, inv, num_segments=n)
            cnt = jax.ops.segment_sum(jnp.ones(n, xyz_b.dtype), inv, num_segments=n)
            cent = sums / jnp.maximum(cnt, 1.0)[:, None]
            return cent, seg[-1] + 1

        def _adapt_voxel_size(xyz, npoint):
            n = xyz.shape[1]
            target_ratio = n / npoint

            def cond(st):
                i, scale, serr, vs, done = st
                return (i < MAX_ITER) & jnp.logical_not(done)

            def body(st):
                i, scale, serr, _, _ = st
                vs = V0 * jnp.exp(scale)
                m = jnp.max(jax.vmap(_count_unique)(_keys(xyz, vs)))
                err = target_ratio - n / m
                serr = serr + err
                diff = KP * err + KI * serr
                scale = scale + 0.01 * (jax.nn.sigmoid(diff) - 0.5)
                done = jnp.abs(m - npoint) <= npoint * 0.1
                return (i + 1, scale, serr, vs, done)

            st0 = (jnp.int32(0), jnp.float32(0.0), jnp.float32(0.0),
                   jnp.float32(V0), jnp.array(False))
            _, _, _, vs, _ = jax.lax.while_loop(cond, body, st0)
            return vs

        vs = _adapt_voxel_size(jax.lax.stop_gradient(xyz), npoint)
        keys = _keys(xyz, vs)
        cent, m = jax.vmap(_centroids)(xyz, keys)
        mmax = int(jnp.max(m))
        x2 = jnp.sum(xyz * xyz, axis=-1)
        c2 = jnp.sum(cent * cent, axis=-1)

        return (np.asarray(cent), np.asarray(x2), np.asarray(c2), mmax)


def _host_refine(xyz_np, x2_np, cent_np, c2_np, cand_idx):
    """Exact winner among candidate point indices per column.

    cand_idx: [JCOLS, K] int64 candidate point indices for one batch.
    Returns [JCOLS] int32: reference-equal argmin (first occurrence ties).
    Distances are evaluated with the reference's formula via jax-CPU so
    rounding matches the oracle's einsum path.
    """
    import jax
    import jax.numpy as jnp

    cpu = jax.devices("cpu")[0]
    with jax.default_device(cpu):
        xyz = jnp.asarray(xyz_np)            # [N, 3]
        x2 = jnp.asarray(x2_np)              # [N]
        cent = jnp.asarray(cent_np)          # [JCOLS, 3]
        c2 = jnp.asarray(c2_np)              # [JCOLS]
        cand = jnp.asarray(cand_idx)         # [JCOLS, K]

        xs = xyz[cand]                       # [JCOLS, K, 3]
        dots = jnp.einsum('kmc,kc->km', xs, cent)  # [JCOLS, K]
        d = (x2[cand] - 2.0 * dots) + c2[:, None]
        d = np.asarray(d)

    mind = d.min(axis=1, keepdims=True)
    big = np.int64(1) << 40
    idxm = np.where(d == mind, cand_idx, big)
    return idxm.min(axis=1).astype(np.int32)


# ------------------------------------------------------------- device kernel
def _build_device_kernel():
    """Raw-bass pipeline with explicit semaphores.

    Engine threads (one chunk = 128 centroids x CS points):
      PE:  4 fp32 matmuls (K=4: [2c;1]^T [x;-x2]) -> psum slot c%2
      ACT: evacuate psum -> SBUF tsb slot c%3
      DVE: Max8 + MaxIndex on tsb -> mv/iv slot c%4
      SP:  DMA mv/iv to DRAM
    """
    import concourse.bass as bass
    import concourse.mybir as mybir
    from contextlib import ExitStack

    f32 = mybir.dt.float32
    u16 = mybir.dt.uint16

    NCHUNK = NBLK * NCH  # 64 chunks total

    nc = bass.Bass(trn_type="TRN2", target_bir_lowering=False, debug=False)
    pts = nc.dram_tensor("pts", [4, N + JSLICE], f32, kind="ExternalInput").ap()
    maxv = nc.dram_tensor("maxv", [NBLK, NCH, 128, 8], f32,
                          kind="ExternalOutput").ap()
    idxv = nc.dram_tensor("idxv", [NBLK, NCH, 128, 8], u16,
                          kind="ExternalOutput").ap()

    with ExitStack() as ctx:
        pts_sb = ctx.enter_context(nc.sbuf_tensor("pts_sb", [4, N + JSLICE], f32))
        tsb = [ctx.enter_context(nc.sbuf_tensor(f"tsb{i}", [128, CS], f32))
               for i in range(3)]
        mv = [ctx.enter_context(nc.sbuf_tensor(f"mv{i}", [128, 8], f32))
              for i in range(4)]
        iv = [ctx.enter_context(nc.sbuf_tensor(f"iv{i}", [128, 8], u16))
              for i in range(4)]
        scr = ctx.enter_context(nc.sbuf_tensor("scr", [128, 8], f32))
        ps = [ctx.enter_context(nc.psum_tensor(f"ps{i}", [128, CS], f32))
              for i in range(2)]
        dma_in = ctx.enter_context(nc.semaphore("dma_in"))
        mm_done = ctx.enter_context(nc.semaphore("mm_done"))
        evac_done = ctx.enter_context(nc.semaphore("evac_done"))
        dve_done = ctx.enter_context(nc.semaphore("dve_done"))
        out_done = ctx.enter_context(nc.semaphore("out_done"))
        block = ctx.enter_context(nc.Block())

        @block.sync
        def _(sync):
            sync.dma_start(pts_sb[:], pts[:]).then_inc(dma_in, 16)
            for c in range(NCHUNK):
                blk, ch = divmod(c, NCH)
                sync.wait_ge(dve_done, c + 1)
                sync.dma_start(maxv[blk, ch], mv[c % 4][:]).then_inc(out_done, 16)
                sync.dma_start(idxv[blk, ch], iv[c % 4][:]).then_inc(out_done, 16)

        @block.tensor
        def _(tensor):
            tensor.wait_ge(dma_in, 16)
            for c in range(NCHUNK):
                blk, ch = divmod(c, NCH)
                if c >= 2:
                    tensor.wait_ge(evac_done, c - 1)
                lhs = pts_sb[:, N + blk * 128:N + (blk + 1) * 128]
                for m in range(CS // MM_N):
                    col = ch * CS + m * MM_N
                    mmi = nc.tensor.matmul(
                        ps[c % 2][:, m * MM_N:(m + 1) * MM_N],
                        lhsT=lhs,
                        rhs=pts_sb[:, col:col + MM_N],
                        start=True, stop=True)
                mmi.then_inc(mm_done, 1)

        @block.scalar
        def _(scalar):
            for c in range(NCHUNK):
                scalar.wait_ge(mm_done, c + 1)
                if c >= 3:
                    scalar.wait_ge(dve_done, c - 2)
                nc.scalar.copy(tsb[c % 3][:], ps[c % 2][:]).then_inc(evac_done, 1)

        @block.vector
        def _(vector):
            for c in range(NCHUNK):
                vector.wait_ge(evac_done, c + 1)
                if c >= 4:
                    vector.wait_ge(out_done, 32 * (c - 3))
                nc.vector.max(out=mv[c % 4][:], in_=tsb[c % 3][:])
                # Hazard spacer: MaxIndex issued back-to-back after Max reads
                # stale in_max on HW; any intervening DVE op fixes it.
                nc.vector.tensor_copy(scr[:], mv[c % 4][:])
                nc.vector.max_index(
                    out=iv[c % 4][:], in_max=mv[c % 4][:],
                    in_values=tsb[c % 3][:]).then_inc(dve_done, 1)

    return nc


def _run_device(pts_list, cw_list):
    """pts_list/cw_list: per-core input arrays. Returns list of result dicts."""
    from concourse import bass_utils

    if "nc" not in _CACHE:
        _CACHE["nc"] = _build_device_kernel()
    nc = _CACHE["nc"]

    in_maps = [{"pts": np.concatenate([p, c], axis=1)}
               for p, c in zip(pts_list, cw_list)]
    res = bass_utils.run_bass_kernel_spmd(
        nc, in_maps, core_ids=list(range(8)), trace=False,
    )
    return res


# ---------------------------------------------------------------- entry point
def kernel(xyz, npoint):
    xyz = np.asarray(xyz, dtype=np.float32)
    npoint_i = int(npoint)
    b, n, _ = xyz.shape
    assert (b, n) == (B, N) and npoint_i == NPOINT, "unexpected problem size"

    cent, x2, c2, mmax = _host_parts(xyz, npoint_i)

    # Per-core device inputs.
    pts_list, cw_list = [], []
    for bb in range(B):
        p = np.empty((4, N), np.float32)
        p[0:3] = xyz[bb].T
        p[3] = -x2[bb]
        centb = cent[bb, :NPOINT]                    # [4096, 3]
        w = np.empty((4, NPOINT), np.float32)
        w[0:3] = 2.0 * centb.T
        w[3] = 1.0
        for s in range(4):
            pts_list.append(p)
            cw_list.append(w[:, s * JSLICE:(s + 1) * JSLICE])

    res = _run_device(pts_list, cw_list)
    results = res.results
    _CACHE["last_inputs"] = (pts_list, cw_list)

    # Host combine: gather candidates, exact-refine per batch.
    nearest = np.empty((B, NPOINT), np.int32)
    for bb in range(B):
        cand = np.empty((NPOINT, NCH * 8), np.int64)
        for s in range(4):
            r = results[bb * 4 + s]
            iv = r["idxv"].astype(np.int64)          # [NBLK, NCH, 128, 8]
            # global point index = ch*CS + local
            iv = iv + (np.arange(NCH, dtype=np.int64) * CS)[None, :, None, None]
            # column j (within slice) = blk*128 + p
            iv = iv.transpose(0, 2, 1, 3).reshape(JSLICE, NCH * 8)
            cand[s * JSLICE:(s + 1) * JSLICE] = iv
        nearest[bb] = _host_refine(xyz[bb], x2[bb], cent[bb, :NPOINT],
                                   c2[bb, :NPOINT], cand)

    pos = np.minimum(np.arange(npoint_i), mmax - 1)
    return nearest[:, pos]


# revision 16
# speedup vs baseline: 5.7064x; 5.7064x over previous
"""AVSNet adaptive voxel sampling kernel for Trainium2 (8 NeuronCores).

Pipeline:
  host (tiny, control-heavy):  PI-control voxel-size adaptation, voxel hashing,
      sort-based centroid computation  (exact replica of the reference, run on
      jax-CPU so float semantics match bit-for-bit).
  device (heavy):  for each of the first 4096 voxel-centroid columns of each
      batch, argmin over all 16384 points of squared distance.  Computed as
      argmax over points of t' = 2<c_j, x_i> - |x_i|^2  (the +|c_j|^2 term is
      constant per column and cannot change the argmin ordering beyond ulp-level
      coincidences).  One K=4 fp32 matmul per tile produces t' directly in PSUM
      (weights [2*c; 1], moving operand [x; -|x|^2]); Max8 + MaxIndex extract
      per-chunk top-8 values and first-occurrence indices.
  host (combine): exact re-evaluation of the ~64 candidate points per column
      with the reference's own formula, first-occurrence tie-breaking.

Sharding: core c -> batch c//4, centroid columns (c%4)*1024 ... +1024.
"""

import os
import sys

import numpy as np

if "/opt/trn_rl_repo" not in sys.path:
    sys.path.insert(0, "/opt/trn_rl_repo")

# ---------------------------------------------------------------- constants
V0 = 0.02
KP = 0.5
KI = 0.1
MAX_ITER = 10
B, N = 2, 16384
NPOINT = 4096

JSLICE = 1024            # centroid columns per core
NBLK = JSLICE // 128     # partition blocks per core
CS = 2048                # point-chunk size per Max8/MaxIndex call
NCH = N // CS            # chunks per block
MM_N = 512               # moving-operand width per fp32 matmul

_CACHE = {}


# ------------------------------------------------------------ host reference
# Verbatim replicas of the reference's control-heavy parts, executed on the
# jax CPU backend so rounding matches the oracle exactly.

def _host_parts(xyz_np, npoint):
    import jax
    import jax.numpy as jnp

    cpu = jax.devices("cpu")[0]
    with jax.default_device(cpu):
        xyz = jnp.asarray(xyz_np)

        def _keys(xyz, voxel_size):
            c = jnp.floor(xyz / voxel_size).astype(jnp.int32)
            return c[..., 0] * 1_000_000 + c[..., 1] * 1_000 + c[..., 2]

        def _count_unique(k):
            sk = jnp.sort(k)
            return 1 + jnp.sum((sk[1:] != sk[:-1]).astype(jnp.int32))

        def _centroids(xyz_b, k):
            n = k.shape[0]
            order = jnp.argsort(k)
            sk = k[order]
            is_new = jnp.concatenate([jnp.ones((1,), bool), sk[1:] != sk[:-1]])
            seg = jnp.cumsum(is_new.astype(jnp.int32)) - 1
            inv = jnp.zeros(n, jnp.int32).at[order].set(seg)
            sums = jax.ops.segment_sum(xyz_b, inv, num_segments=n)
            cnt = jax.ops.segment_sum(jnp.ones(n, xyz_b.dtype), inv, num_segments=n)
            cent = sums / jnp.maximum(cnt, 1.0)[:, None]
            return cent, seg[-1] + 1

        def _adapt_voxel_size(xyz, npoint):
            n = xyz.shape[1]
            target_ratio = n / npoint

            def cond(st):
                i, scale, serr, vs, done = st
                return (i < MAX_ITER) & jnp.logical_not(done)

            def body(st):
                i, scale, serr, _, _ = st
                vs = V0 * jnp.exp(scale)
                m = jnp.max(jax.vmap(_count_unique)(_keys(xyz, vs)))
                err = target_ratio - n / m
                serr = serr + err
                diff = KP * err + KI * serr
                scale = scale + 0.01 * (jax.nn.sigmoid(diff) - 0.5)
                done = jnp.abs(m - npoint) <= npoint * 0.1
                return (i + 1, scale, serr, vs, done)

            st0 = (jnp.int32(0), jnp.float32(0.0), jnp.float32(0.0),
                   jnp.float32(V0), jnp.array(False))
            _, _, _, vs, _ = jax.lax.while_loop(cond, body, st0)
            return vs

        vs = _adapt_voxel_size(jax.lax.stop_gradient(xyz), npoint)
        keys = _keys(xyz, vs)
        cent, m = jax.vmap(_centroids)(xyz, keys)
        mmax = int(jnp.max(m))
        x2 = jnp.sum(xyz * xyz, axis=-1)
        c2 = jnp.sum(cent * cent, axis=-1)

        return (np.asarray(cent), np.asarray(x2), np.asarray(c2), mmax)


def _host_refine(xyz_np, x2_np, cent_np, c2_np, cand_idx):
    """Exact winner among candidate point indices per column.

    cand_idx: [JCOLS, K] int64 candidate point indices for one batch.
    Returns [JCOLS] int32: reference-equal argmin (first occurrence ties).
    Distances are evaluated with the reference's formula via jax-CPU so
    rounding matches the oracle's einsum path.
    """
    import jax
    import jax.numpy as jnp

    cpu = jax.devices("cpu")[0]
    with jax.default_device(cpu):
        xyz = jnp.asarray(xyz_np)            # [N, 3]
        x2 = jnp.asarray(x2_np)              # [N]
        cent = jnp.asarray(cent_np)          # [JCOLS, 3]
        c2 = jnp.asarray(c2_np)              # [JCOLS]
        cand = jnp.asarray(cand_idx)         # [JCOLS, K]

        xs = xyz[cand]                       # [JCOLS, K, 3]
        dots = jnp.einsum('kmc,kc->km', xs, cent)  # [JCOLS, K]
        d = (x2[cand] - 2.0 * dots) + c2[:, None]
        d = np.asarray(d)

    mind = d.min(axis=1, keepdims=True)
    big = np.int64(1) << 40
    idxm = np.where(d == mind, cand_idx, big)
    return idxm.min(axis=1).astype(np.int32)


# ------------------------------------------------------------- device kernel
def _build_device_kernel():
    """Raw-bass pipeline with explicit semaphores.

    Engine threads (one chunk = 128 centroids x CS points):
      PE:  4 fp32 matmuls (K=4: [2c;1]^T [x;-x2]) -> psum slot c%2
      ACT: evacuate psum -> SBUF tsb slot c%3
      DVE: Max8 + MaxIndex on tsb -> mv/iv slot c%4
      SP:  DMA mv/iv to DRAM
    """
    import concourse.bass as bass
    import concourse.mybir as mybir
    from contextlib import ExitStack

    f32 = mybir.dt.float32
    u16 = mybir.dt.uint16

    NCHUNK = NBLK * NCH  # 64 chunks total

    nc = bass.Bass(trn_type="TRN2", target_bir_lowering=False, debug=False)
    pts = nc.dram_tensor("pts", [4, N + JSLICE], f32, kind="ExternalInput").ap()
    maxv = nc.dram_tensor("maxv", [NBLK, NCH, 128, 8], f32,
                          kind="ExternalOutput").ap()
    idxv = nc.dram_tensor("idxv", [NBLK, NCH, 128, 8], u16,
                          kind="ExternalOutput").ap()

    with ExitStack() as ctx:
        pts_sb = ctx.enter_context(nc.sbuf_tensor("pts_sb", [4, N + JSLICE], f32))
        tsb = [ctx.enter_context(nc.sbuf_tensor(f"tsb{i}", [128, CS], f32))
               for i in range(3)]
        mv = [ctx.enter_context(nc.sbuf_tensor(f"mv{i}", [128, 8], f32))
              for i in range(4)]
        iv = [ctx.enter_context(nc.sbuf_tensor(f"iv{i}", [128, 8], u16))
              for i in range(4)]
        scr = ctx.enter_context(nc.sbuf_tensor("scr", [128, 8], f32))
        ps = [ctx.enter_context(nc.psum_tensor(f"ps{i}", [128, CS], f32))
              for i in range(2)]
        dma_in = ctx.enter_context(nc.semaphore("dma_in"))
        mm_done = ctx.enter_context(nc.semaphore("mm_done"))
        evac_done = ctx.enter_context(nc.semaphore("evac_done"))
        dve_done = ctx.enter_context(nc.semaphore("dve_done"))
        out_done = ctx.enter_context(nc.semaphore("out_done"))
        block = ctx.enter_context(nc.Block())

        @block.sync
        def _(sync):
            sync.dma_start(pts_sb[:], pts[:]).then_inc(dma_in, 16)
            for c in range(NCHUNK):
                blk, ch = divmod(c, NCH)
                sync.wait_ge(dve_done, c + 1)
                sync.dma_start(maxv[blk, ch], mv[c % 4][:]).then_inc(out_done, 16)
                sync.dma_start(idxv[blk, ch], iv[c % 4][:]).then_inc(out_done, 16)

        @block.tensor
        def _(tensor):
            tensor.wait_ge(dma_in, 16)
            for c in range(NCHUNK):
                blk, ch = divmod(c, NCH)
                if c >= 2:
                    tensor.wait_ge(evac_done, c - 1)
                lhs = pts_sb[:, N + blk * 128:N + (blk + 1) * 128]
                for m in range(CS // MM_N):
                    col = ch * CS + m * MM_N
                    mmi = nc.tensor.matmul(
                        ps[c % 2][:, m * MM_N:(m + 1) * MM_N],
                        lhsT=lhs,
                        rhs=pts_sb[:, col:col + MM_N],
                        start=True, stop=True)
                mmi.then_inc(mm_done, 1)

        @block.scalar
        def _(scalar):
            for c in range(NCHUNK):
                scalar.wait_ge(mm_done, c + 1)
                if c >= 3:
                    scalar.wait_ge(dve_done, c - 2)
                nc.scalar.copy(tsb[c % 3][:], ps[c % 2][:]).then_inc(evac_done, 1)

        @block.vector
        def _(vector):
            for c in range(NCHUNK):
                vector.wait_ge(evac_done, c + 1)
                if c >= 4:
                    vector.wait_ge(out_done, 32 * (c - 3))
                nc.vector.max(out=mv[c % 4][:], in_=tsb[c % 3][:])
                # Hazard spacer: MaxIndex issued back-to-back after Max reads
                # stale in_max on HW; any intervening DVE op fixes it.
                nc.vector.tensor_copy(scr[:], mv[c % 4][:])
                nc.vector.max_index(
                    out=iv[c % 4][:], in_max=mv[c % 4][:],
                    in_values=tsb[c % 3][:]).then_inc(dve_done, 1)

    return nc


def _get_runner():
    """Build (once) a cached jitted SPMD executor over 8 cores.

    Returns (fn, out_names, out_avals): fn takes the concatenated input
    array [8*4, N+JSLICE] and returns the tuple of sharded output arrays.
    """
    if "runner" in _CACHE:
        return _CACHE["runner"]

    import jax
    import concourse.mybir as mybir
    from jax.sharding import Mesh, PartitionSpec
    from jax.experimental.shard_map import shard_map
    from concourse import bass2jax

    nc = _CACHE.get("nc")
    if nc is None:
        nc = _CACHE["nc"] = _build_device_kernel()

    bass2jax.install_neuronx_cc_hook()

    partition_name = (nc.partition_id_tensor.name
                      if nc.partition_id_tensor else None)
    in_names, out_names, out_avals = [], [], []
    for alloc in nc.m.functions[0].allocations:
        if not isinstance(alloc, mybir.MemoryLocationSet):
            continue
        name = alloc.memorylocations[0].name
        if alloc.kind == "ExternalInput":
            if name != partition_name:
                in_names.append(name)
        elif alloc.kind == "ExternalOutput":
            out_names.append(name)
            out_avals.append(jax.core.ShapedArray(
                tuple(alloc.tensor_shape), mybir.dt.np(alloc.dtype)))
    assert in_names == ["pts"], in_names
    n_params = 1
    n_outs = len(out_avals)
    all_in_names = in_names + out_names
    if partition_name is not None:
        all_in_names.append(partition_name)
    donate = tuple(range(n_params, n_params + n_outs))

    def _body(*args):
        operands = list(args)
        if partition_name is not None:
            operands.append(bass2jax.partition_id_tensor())
        outs = bass2jax._bass_exec_p.bind(
            *operands,
            out_avals=tuple(out_avals),
            in_names=tuple(all_in_names),
            out_names=tuple(out_names),
            lowering_input_output_aliases=(),
            sim_require_finite=True,
            sim_require_nnan=True,
            nc=nc,
        )
        return tuple(outs)

    devices = jax.devices()[:8]
    mesh = Mesh(np.asarray(devices), ("core",))
    in_specs = (PartitionSpec("core"),) * (n_params + n_outs)
    out_specs = (PartitionSpec("core"),) * n_outs
    fn = jax.jit(
        shard_map(_body, mesh=mesh, in_specs=in_specs, out_specs=out_specs,
                  check_rep=False),
        donate_argnums=donate, keep_unused=True)

    _CACHE["runner"] = (fn, out_names, out_avals)
    return _CACHE["runner"]


def _run_device(pts_list, cw_list):
    """pts_list/cw_list: per-core input arrays. Returns list of result dicts."""
    fn, out_names, out_avals = _get_runner()
    concat_in = np.concatenate(
        [np.concatenate([p, c], axis=1) for p, c in zip(pts_list, cw_list)],
        axis=0)
    zeros = [np.zeros((8 * a.shape[0], *a.shape[1:]), a.dtype) for a in out_avals]
    out_arrs = fn(concat_in, *zeros)
    results = [
        {name: np.asarray(out_arrs[i]).reshape(8, *out_avals[i].shape)[c]
         for i, name in enumerate(out_names)}
        for c in range(8)
    ]

    class _R:  # minimal result shim
        pass
    r = _R()
    r.results = results
    return r


# ---------------------------------------------------------------- entry point
def kernel(xyz, npoint):
    xyz = np.asarray(xyz, dtype=np.float32)
    npoint_i = int(npoint)
    b, n, _ = xyz.shape
    assert (b, n) == (B, N) and npoint_i == NPOINT, "unexpected problem size"

    cent, x2, c2, mmax = _host_parts(xyz, npoint_i)

    # Per-core device inputs.
    pts_list, cw_list = [], []
    for bb in range(B):
        p = np.empty((4, N), np.float32)
        p[0:3] = xyz[bb].T
        p[3] = -x2[bb]
        centb = cent[bb, :NPOINT]                    # [4096, 3]
        w = np.empty((4, NPOINT), np.float32)
        w[0:3] = 2.0 * centb.T
        w[3] = 1.0
        for s in range(4):
            pts_list.append(p)
            cw_list.append(w[:, s * JSLICE:(s + 1) * JSLICE])

    res = _run_device(pts_list, cw_list)
    results = res.results
    _CACHE["last_inputs"] = (pts_list, cw_list)

    # Host combine: gather candidates, exact-refine per batch.
    nearest = np.empty((B, NPOINT), np.int32)
    for bb in range(B):
        cand = np.empty((NPOINT, NCH * 8), np.int64)
        for s in range(4):
            r = results[bb * 4 + s]
            iv = r["idxv"].astype(np.int64)          # [NBLK, NCH, 128, 8]
            # global point index = ch*CS + local
            iv = iv + (np.arange(NCH, dtype=np.int64) * CS)[None, :, None, None]
            # column j (within slice) = blk*128 + p
            iv = iv.transpose(0, 2, 1, 3).reshape(JSLICE, NCH * 8)
            cand[s * JSLICE:(s + 1) * JSLICE] = iv
        nearest[bb] = _host_refine(xyz[bb], x2[bb], cent[bb, :NPOINT],
                                   c2[bb, :NPOINT], cand)

    pos = np.minimum(np.arange(npoint_i), mmax - 1)
    return nearest[:, pos]


# revision 18
# speedup vs baseline: 3243.9270x; 568.4728x over previous
"""AVSNet adaptive voxel sampling kernel for Trainium2 (8 NeuronCores).

Pipeline:
  host (tiny, control-heavy):  PI-control voxel-size adaptation, voxel hashing,
      sort-based centroid computation  (exact replica of the reference, run on
      jax-CPU so float semantics match bit-for-bit).
  device (heavy):  for each of the first 4096 voxel-centroid columns of each
      batch, argmin over all 16384 points of squared distance.  Computed as
      argmax over points of t' = 2<c_j, x_i> - |x_i|^2  (the +|c_j|^2 term is
      constant per column and cannot change the argmin ordering beyond ulp-level
      coincidences).  One K=4 fp32 matmul per tile produces t' directly in PSUM
      (weights [2*c; 1], moving operand [x; -|x|^2]); Max8 + MaxIndex extract
      per-chunk top-8 values and first-occurrence indices.
  host (combine): exact re-evaluation of the ~64 candidate points per column
      with the reference's own formula, first-occurrence tie-breaking.

Sharding: core c -> batch c//4, centroid columns (c%4)*1024 ... +1024.
"""

import os
import sys

import numpy as np

if "/opt/trn_rl_repo" not in sys.path:
    sys.path.insert(0, "/opt/trn_rl_repo")

# ---------------------------------------------------------------- constants
V0 = 0.02
KP = 0.5
KI = 0.1
MAX_ITER = 10
B, N = 2, 16384
NPOINT = 4096

JSLICE = 1024            # centroid columns per core
NBLK = JSLICE // 128     # partition blocks per core
CS = 2048                # point-chunk size per Max8/MaxIndex call
NCH = N // CS            # chunks per block
MM_N = 512               # moving-operand width per fp32 matmul

_CACHE = {}


# ------------------------------------------------------------ host reference
# Verbatim replicas of the reference's control-heavy parts, executed on the
# jax CPU backend so rounding matches the oracle exactly.

def _host_parts(xyz_np, npoint):
    import jax
    import jax.numpy as jnp

    cpu = jax.devices("cpu")[0]
    with jax.default_device(cpu):
        xyz = jnp.asarray(xyz_np)

        def _keys(xyz, voxel_size):
            c = jnp.floor(xyz / voxel_size).astype(jnp.int32)
            return c[..., 0] * 1_000_000 + c[..., 1] * 1_000 + c[..., 2]

        def _count_unique(k):
            sk = jnp.sort(k)
            return 1 + jnp.sum((sk[1:] != sk[:-1]).astype(jnp.int32))

        def _centroids(xyz_b, k):
            n = k.shape[0]
            order = jnp.argsort(k)
            sk = k[order]
            is_new = jnp.concatenate([jnp.ones((1,), bool), sk[1:] != sk[:-1]])
            seg = jnp.cumsum(is_new.astype(jnp.int32)) - 1
            inv = jnp.zeros(n, jnp.int32).at[order].set(seg)
            sums = jax.ops.segment_sum(xyz_b, inv, num_segments=n)
            cnt = jax.ops.segment_sum(jnp.ones(n, xyz_b.dtype), inv, num_segments=n)
            cent = sums / jnp.maximum(cnt, 1.0)[:, None]
            return cent, seg[-1] + 1

        def _adapt_voxel_size(xyz, npoint):
            n = xyz.shape[1]
            target_ratio = n / npoint

            def cond(st):
                i, scale, serr, vs, done = st
                return (i < MAX_ITER) & jnp.logical_not(done)

            def body(st):
                i, scale, serr, _, _ = st
                vs = V0 * jnp.exp(scale)
                m = jnp.max(jax.vmap(_count_unique)(_keys(xyz, vs)))
                err = target_ratio - n / m
                serr = serr + err
                diff = KP * err + KI * serr
                scale = scale + 0.01 * (jax.nn.sigmoid(diff) - 0.5)
                done = jnp.abs(m - npoint) <= npoint * 0.1
                return (i + 1, scale, serr, vs, done)

            st0 = (jnp.int32(0), jnp.float32(0.0), jnp.float32(0.0),
                   jnp.float32(V0), jnp.array(False))
            _, _, _, vs, _ = jax.lax.while_loop(cond, body, st0)
            return vs

        vs = _adapt_voxel_size(jax.lax.stop_gradient(xyz), npoint)
        keys = _keys(xyz, vs)
        cent, m = jax.vmap(_centroids)(xyz, keys)
        mmax = int(jnp.max(m))
        x2 = jnp.sum(xyz * xyz, axis=-1)
        c2 = jnp.sum(cent * cent, axis=-1)

        return (np.asarray(cent), np.asarray(x2), np.asarray(c2), mmax)


def _host_refine(xyz_np, x2_np, cent_np, c2_np, cand_idx):
    """Exact winner among candidate point indices per column.

    cand_idx: [JCOLS, K] int64 candidate point indices for one batch.
    Returns [JCOLS] int32: reference-equal argmin (first occurrence ties).
    Distances are evaluated with the reference's formula via jax-CPU so
    rounding matches the oracle's einsum path.
    """
    import jax
    import jax.numpy as jnp

    cpu = jax.devices("cpu")[0]
    with jax.default_device(cpu):
        xyz = jnp.asarray(xyz_np)            # [N, 3]
        x2 = jnp.asarray(x2_np)              # [N]
        cent = jnp.asarray(cent_np)          # [JCOLS, 3]
        c2 = jnp.asarray(c2_np)              # [JCOLS]
        cand = jnp.asarray(cand_idx)         # [JCOLS, K]

        xs = xyz[cand]                       # [JCOLS, K, 3]
        dots = jnp.einsum('kmc,kc->km', xs, cent)  # [JCOLS, K]
        d = (x2[cand] - 2.0 * dots) + c2[:, None]
        d = np.asarray(d)

    mind = d.min(axis=1, keepdims=True)
    big = np.int64(1) << 40
    idxm = np.where(d == mind, cand_idx, big)
    return idxm.min(axis=1).astype(np.int32)


# ------------------------------------------------------------- device kernel
def _build_device_kernel(repeat=1):
    """Raw-bass pipeline with explicit semaphores.

    Engine threads (one chunk = 128 centroids x CS points):
      PE:  4 fp32 matmuls (K=4: [2c;1]^T [x;-x2]) -> psum slot c%2
      ACT: evacuate psum -> SBUF tsb slot c%3
      DVE: Max8 + MaxIndex on tsb -> mv/iv slot c%4
      SP:  DMA mv/iv to DRAM
    """
    import concourse.bass as bass
    import concourse.mybir as mybir
    from contextlib import ExitStack

    f32 = mybir.dt.float32
    u16 = mybir.dt.uint16

    NCHUNK = NBLK * NCH  # 64 chunks total

    nc = bass.Bass(trn_type="TRN2", target_bir_lowering=False, debug=False)
    pts = nc.dram_tensor("pts", [4, N + JSLICE], f32, kind="ExternalInput").ap()
    maxv = nc.dram_tensor("maxv", [NBLK, NCH, 128, 8], f32,
                          kind="ExternalOutput").ap()
    idxv = nc.dram_tensor("idxv", [NBLK, NCH, 128, 8], u16,
                          kind="ExternalOutput").ap()

    with ExitStack() as ctx:
        pts_sb = ctx.enter_context(nc.sbuf_tensor("pts_sb", [4, N + JSLICE], f32))
        tsb = [ctx.enter_context(nc.sbuf_tensor(f"tsb{i}", [128, CS], f32))
               for i in range(3)]
        mv = [ctx.enter_context(nc.sbuf_tensor(f"mv{i}", [128, 8], f32))
              for i in range(4)]
        iv = [ctx.enter_context(nc.sbuf_tensor(f"iv{i}", [128, 8], u16))
              for i in range(4)]
        scr = ctx.enter_context(nc.sbuf_tensor("scr", [128, 8], f32))
        ps = [ctx.enter_context(nc.psum_tensor(f"ps{i}", [128, CS], f32))
              for i in range(2)]
        dma_in = ctx.enter_context(nc.semaphore("dma_in"))
        mm_done = ctx.enter_context(nc.semaphore("mm_done"))
        evac_done = ctx.enter_context(nc.semaphore("evac_done"))
        dve_done = ctx.enter_context(nc.semaphore("dve_done"))
        out_done = ctx.enter_context(nc.semaphore("out_done"))
        block = ctx.enter_context(nc.Block())

        NTOT = repeat * NCHUNK

        @block.sync
        def _(sync):
            sync.dma_start(pts_sb[:], pts[:]).then_inc(dma_in, 16)
            for g in range(NTOT):
                c = g % NCHUNK
                blk, ch = divmod(c, NCH)
                sync.wait_ge(dve_done, g + 1)
                sync.dma_start(maxv[blk, ch], mv[g % 4][:]).then_inc(out_done, 16)
                sync.dma_start(idxv[blk, ch], iv[g % 4][:]).then_inc(out_done, 16)

        @block.tensor
        def _(tensor):
            tensor.wait_ge(dma_in, 16)
            for g in range(NTOT):
                c = g % NCHUNK
                blk, ch = divmod(c, NCH)
                if g >= 2:
                    tensor.wait_ge(evac_done, g - 1)
                lhs = pts_sb[:, N + blk * 128:N + (blk + 1) * 128]
                for m in range(CS // MM_N):
                    col = ch * CS + m * MM_N
                    mmi = nc.tensor.matmul(
                        ps[g % 2][:, m * MM_N:(m + 1) * MM_N],
                        lhsT=lhs,
                        rhs=pts_sb[:, col:col + MM_N],
                        start=True, stop=True)
                mmi.then_inc(mm_done, 1)

        @block.scalar
        def _(scalar):
            for g in range(NTOT):
                scalar.wait_ge(mm_done, g + 1)
                if g >= 3:
                    scalar.wait_ge(dve_done, g - 2)
                nc.scalar.copy(tsb[g % 3][:], ps[g % 2][:]).then_inc(evac_done, 1)

        @block.vector
        def _(vector):
            for g in range(NTOT):
                vector.wait_ge(evac_done, g + 1)
                if g >= 4:
                    vector.wait_ge(out_done, 32 * (g - 3))
                nc.vector.max(out=mv[g % 4][:], in_=tsb[g % 3][:])
                # Hazard spacer: MaxIndex issued back-to-back after Max reads
                # stale in_max on HW; any intervening DVE op fixes it.
                nc.vector.tensor_copy(scr[:], mv[g % 4][:])
                nc.vector.max_index(
                    out=iv[g % 4][:], in_max=mv[g % 4][:],
                    in_values=tsb[g % 3][:]).then_inc(dve_done, 1)

    return nc


def _get_runner():
    """Build (once) a cached jitted SPMD executor over 8 cores.

    Returns (fn, out_names, out_avals): fn takes the concatenated input
    array [8*4, N+JSLICE] and returns the tuple of sharded output arrays.
    """
    if "runner" in _CACHE:
        return _CACHE["runner"]

    import jax
    import concourse.mybir as mybir
    from jax.sharding import Mesh, PartitionSpec
    from jax.experimental.shard_map import shard_map
    from concourse import bass2jax

    nc = _CACHE.get("nc")
    if nc is None:
        nc = _CACHE["nc"] = _build_device_kernel()

    bass2jax.install_neuronx_cc_hook()

    partition_name = (nc.partition_id_tensor.name
                      if nc.partition_id_tensor else None)
    in_names, out_names, out_avals = [], [], []
    for alloc in nc.m.functions[0].allocations:
        if not isinstance(alloc, mybir.MemoryLocationSet):
            continue
        name = alloc.memorylocations[0].name
        if alloc.kind == "ExternalInput":
            if name != partition_name:
                in_names.append(name)
        elif alloc.kind == "ExternalOutput":
            out_names.append(name)
            out_avals.append(jax.core.ShapedArray(
                tuple(alloc.tensor_shape), mybir.dt.np(alloc.dtype)))
    assert in_names == ["pts"], in_names
    n_params = 1
    n_outs = len(out_avals)
    all_in_names = in_names + out_names
    if partition_name is not None:
        all_in_names.append(partition_name)
    donate = tuple(range(n_params, n_params + n_outs))

    def _body(*args):
        operands = list(args)
        if partition_name is not None:
            operands.append(bass2jax.partition_id_tensor())
        outs = bass2jax._bass_exec_p.bind(
            *operands,
            out_avals=tuple(out_avals),
            in_names=tuple(all_in_names),
            out_names=tuple(out_names),
            lowering_input_output_aliases=(),
            sim_require_finite=True,
            sim_require_nnan=True,
            nc=nc,
        )
        return tuple(outs)

    devices = jax.devices()[:8]
    mesh = Mesh(np.asarray(devices), ("core",))
    in_specs = (PartitionSpec("core"),) * (n_params + n_outs)
    out_specs = (PartitionSpec("core"),) * n_outs
    fn = jax.jit(
        shard_map(_body, mesh=mesh, in_specs=in_specs, out_specs=out_specs,
                  check_rep=False),
        donate_argnums=donate, keep_unused=True)

    _CACHE["runner"] = (fn, out_names, out_avals)
    return _CACHE["runner"]


def _run_device(pts_list, cw_list):
    """pts_list/cw_list: per-core input arrays. Returns list of result dicts."""
    fn, out_names, out_avals = _get_runner()
    concat_in = np.concatenate(
        [np.concatenate([p, c], axis=1) for p, c in zip(pts_list, cw_list)],
        axis=0)
    zeros = [np.zeros((8 * a.shape[0], *a.shape[1:]), a.dtype) for a in out_avals]
    out_arrs = fn(concat_in, *zeros)
    results = [
        {name: np.asarray(out_arrs[i]).reshape(8, *out_avals[i].shape)[c]
         for i, name in enumerate(out_names)}
        for c in range(8)
    ]

    class _R:  # minimal result shim
        pass
    r = _R()
    r.results = results
    return r


# ---------------------------------------------------------------- entry point
def kernel(xyz, npoint):
    xyz = np.asarray(xyz, dtype=np.float32)
    npoint_i = int(npoint)
    b, n, _ = xyz.shape
    assert (b, n) == (B, N) and npoint_i == NPOINT, "unexpected problem size"

    cent, x2, c2, mmax = _host_parts(xyz, npoint_i)

    # Per-core device inputs.
    pts_list, cw_list = [], []
    for bb in range(B):
        p = np.empty((4, N), np.float32)
        p[0:3] = xyz[bb].T
        p[3] = -x2[bb]
        centb = cent[bb, :NPOINT]                    # [4096, 3]
        w = np.empty((4, NPOINT), np.float32)
        w[0:3] = 2.0 * centb.T
        w[3] = 1.0
        for s in range(4):
            pts_list.append(p)
            cw_list.append(w[:, s * JSLICE:(s + 1) * JSLICE])

    res = _run_device(pts_list, cw_list)
    results = res.results
    _CACHE["last_inputs"] = (pts_list, cw_list)

    # Host combine: gather candidates, exact-refine per batch.
    nearest = np.empty((B, NPOINT), np.int32)
    for bb in range(B):
        cand = np.empty((NPOINT, NCH * 8), np.int64)
        for s in range(4):
            r = results[bb * 4 + s]
            iv = r["idxv"].astype(np.int64)          # [NBLK, NCH, 128, 8]
            # global point index = ch*CS + local
            iv = iv + (np.arange(NCH, dtype=np.int64) * CS)[None, :, None, None]
            # column j (within slice) = blk*128 + p
            iv = iv.transpose(0, 2, 1, 3).reshape(JSLICE, NCH * 8)
            cand[s * JSLICE:(s + 1) * JSLICE] = iv
        nearest[bb] = _host_refine(xyz[bb], x2[bb], cent[bb, :NPOINT],
                                   c2[bb, :NPOINT], cand)

    pos = np.minimum(np.arange(npoint_i), mmax - 1)
    return nearest[:, pos]


# revision 30
# speedup vs baseline: 5421.6421x; 1.6713x over previous
"""AVSNet adaptive voxel sampling kernel for Trainium2 (8 NeuronCores).

Pipeline:
  host (tiny, control-heavy):  PI-control voxel-size adaptation, voxel hashing,
      sort-based centroid computation  (exact replica of the reference, run on
      jax-CPU so float semantics match bit-for-bit).
  device (heavy):  for each of the first 4096 voxel-centroid columns of each
      batch, argmin over all 16384 points of squared distance.  Computed as
      argmax over points of t' = 2<c_j, x_i> - |x_i|^2  (the +|c_j|^2 term is
      constant per column and cannot change the argmin ordering beyond ulp-level
      coincidences).  One K=4 fp32 matmul per tile produces t' directly in PSUM
      (weights [2*c; 1], moving operand [x; -|x|^2]); Max8 + MaxIndex extract
      per-chunk top-8 values and first-occurrence indices.
  host (combine): exact re-evaluation of the ~64 candidate points per column
      with the reference's own formula, first-occurrence tie-breaking.

Sharding: core c -> batch c//4, centroid columns (c%4)*1024 ... +1024.
"""

import os
import sys

import numpy as np

if "/opt/trn_rl_repo" not in sys.path:
    sys.path.insert(0, "/opt/trn_rl_repo")

# ---------------------------------------------------------------- constants
V0 = 0.02
KP = 0.5
KI = 0.1
MAX_ITER = 10
B, N = 2, 16384
NPOINT = 4096

JSLICE = 1024            # centroid columns per core
NBLK = JSLICE // 128     # partition blocks per core
CS = 2048                # PSUM chunk size (points)
NCH = N // CS            # psum chunks per block
MM_N = 512               # moving-operand width per matmul (one PSUM bank)
DCS = 8192               # DVE chunk size (points) per Max8/MaxIndex
NDC = N // DCS           # DVE chunks per block
FOLD = DCS // 2048       # positions folded into each MaxIndex slot (4)

_CACHE = {}


# ------------------------------------------------------------ host reference
# Verbatim replicas of the reference's control-heavy parts, executed on the
# jax CPU backend so rounding matches the oracle exactly.

def _host_parts(xyz_np, npoint):
    import jax
    import jax.numpy as jnp

    cpu = jax.devices("cpu")[0]
    with jax.default_device(cpu):
        xyz = jnp.asarray(xyz_np)

        def _keys(xyz, voxel_size):
            c = jnp.floor(xyz / voxel_size).astype(jnp.int32)
            return c[..., 0] * 1_000_000 + c[..., 1] * 1_000 + c[..., 2]

        def _count_unique(k):
            sk = jnp.sort(k)
            return 1 + jnp.sum((sk[1:] != sk[:-1]).astype(jnp.int32))

        def _centroids(xyz_b, k):
            n = k.shape[0]
            order = jnp.argsort(k)
            sk = k[order]
            is_new = jnp.concatenate([jnp.ones((1,), bool), sk[1:] != sk[:-1]])
            seg = jnp.cumsum(is_new.astype(jnp.int32)) - 1
            inv = jnp.zeros(n, jnp.int32).at[order].set(seg)
            sums = jax.ops.segment_sum(xyz_b, inv, num_segments=n)
            cnt = jax.ops.segment_sum(jnp.ones(n, xyz_b.dtype), inv, num_segments=n)
            cent = sums / jnp.maximum(cnt, 1.0)[:, None]
            return cent, seg[-1] + 1

        def _adapt_voxel_size(xyz, npoint):
            n = xyz.shape[1]
            target_ratio = n / npoint

            def cond(st):
                i, scale, serr, vs, done = st
                return (i < MAX_ITER) & jnp.logical_not(done)

            def body(st):
                i, scale, serr, _, _ = st
                vs = V0 * jnp.exp(scale)
                m = jnp.max(jax.vmap(_count_unique)(_keys(xyz, vs)))
                err = target_ratio - n / m
                serr = serr + err
                diff = KP * err + KI * serr
                scale = scale + 0.01 * (jax.nn.sigmoid(diff) - 0.5)
                done = jnp.abs(m - npoint) <= npoint * 0.1
                return (i + 1, scale, serr, vs, done)

            st0 = (jnp.int32(0), jnp.float32(0.0), jnp.float32(0.0),
                   jnp.float32(V0), jnp.array(False))
            _, _, _, vs, _ = jax.lax.while_loop(cond, body, st0)
            return vs

        vs = _adapt_voxel_size(jax.lax.stop_gradient(xyz), npoint)
        keys = _keys(xyz, vs)
        cent, m = jax.vmap(_centroids)(xyz, keys)
        mmax = int(jnp.max(m))
        x2 = jnp.sum(xyz * xyz, axis=-1)
        c2 = jnp.sum(cent * cent, axis=-1)

        return (np.asarray(cent), np.asarray(x2), np.asarray(c2), mmax)


def _host_refine(xyz_np, x2_np, cent_np, c2_np, cand_idx):
    """Exact winner among candidate point indices per column.

    cand_idx: [JCOLS, K] int64 candidate point indices for one batch.
    Returns [JCOLS] int32: reference-equal argmin (first occurrence ties).
    Distances are evaluated with the reference's formula via jax-CPU so
    rounding matches the oracle's einsum path.
    """
    import jax
    import jax.numpy as jnp

    cpu = jax.devices("cpu")[0]
    with jax.default_device(cpu):
        xyz = jnp.asarray(xyz_np)            # [N, 3]
        x2 = jnp.asarray(x2_np)              # [N]
        cent = jnp.asarray(cent_np)          # [JCOLS, 3]
        c2 = jnp.asarray(c2_np)              # [JCOLS]
        cand = jnp.asarray(cand_idx)         # [JCOLS, K]

        xs = xyz[cand]                       # [JCOLS, K, 3]
        dots = jnp.einsum('kmc,kc->km', xs, cent)  # [JCOLS, K]
        d = (x2[cand] - 2.0 * dots) + c2[:, None]
        d = np.asarray(d)

    mind = d.min(axis=1, keepdims=True)
    big = np.int64(1) << 40
    idxm = np.where(d == mind, cand_idx, big)
    return idxm.min(axis=1).astype(np.int32)


# ------------------------------------------------------------- device kernel
def _build_device_kernel(repeat=1):
    """Raw-bass pipeline with explicit semaphores.

    Engine threads (one chunk = 128 centroids x CS points):
      PE:  4 fp32 matmuls (K=4: [2c;1]^T [x;-x2]) -> psum slot c%2
      ACT: evacuate psum -> SBUF tsb slot c%3
      DVE: Max8 + MaxIndex on tsb -> mv/iv slot c%4
      SP:  DMA mv/iv to DRAM
    """
    import concourse.bass as bass
    import concourse.mybir as mybir
    from contextlib import ExitStack

    f32 = mybir.dt.float32
    f16 = mybir.dt.float16
    bf16 = mybir.dt.bfloat16
    u16 = mybir.dt.uint16

    NCHUNK = NBLK * NCH  # 64 chunks total

    nc = bass.Bass(trn_type="TRN2", target_bir_lowering=False, debug=False)
    # fp16 inputs: PE runs 1 cycle/row with cheap FWL weight loads.  Input
    # rounding is absorbed by the host's exact refinement of the candidates.
    pts = nc.dram_tensor("pts", [4, N + JSLICE], f16, kind="ExternalInput").ap()
    idxv = nc.dram_tensor("idxv", [NBLK, NDC, 128, 8], u16,
                          kind="ExternalOutput").ap()

    with ExitStack() as ctx:
        pts_sb = ctx.enter_context(nc.sbuf_tensor("pts_sb", [4, N + JSLICE], f16))
        tsb = [ctx.enter_context(nc.sbuf_tensor(f"tsb{i}", [128, DCS], bf16))
               for i in range(3)]
        f1 = ctx.enter_context(nc.sbuf_tensor("f1", [128, DCS // 2], bf16))
        f2 = ctx.enter_context(nc.sbuf_tensor("f2", [128, DCS // 4], bf16))
        mv = [ctx.enter_context(nc.sbuf_tensor(f"mv{i}", [128, 8], bf16))
              for i in range(4)]
        iv = [ctx.enter_context(nc.sbuf_tensor(f"iv{i}", [128, 8], u16))
              for i in range(4)]
        scr = ctx.enter_context(nc.sbuf_tensor("scr", [128, 8], bf16))
        ps = [ctx.enter_context(nc.psum_tensor(f"ps{i}", [128, CS], f32))
              for i in range(2)]
        dma_in = ctx.enter_context(nc.semaphore("dma_in"))
        mm_done = ctx.enter_context(nc.semaphore("mm_done"))
        evac_done = ctx.enter_context(nc.semaphore("evac_done"))
        dve_done = ctx.enter_context(nc.semaphore("dve_done"))
        out_done = ctx.enter_context(nc.semaphore("out_done"))
        block = ctx.enter_context(nc.Block())

        NTOT = repeat * NCHUNK          # psum chunks overall
        NDTOT = repeat * NBLK * NDC     # DVE chunks overall
        QP = DCS // CS                  # psum chunks per DVE chunk (4)

        @block.sync
        def _(sync):
            sync.dma_start(pts_sb[:], pts[:]).then_inc(dma_in, 16)
            for d in range(NDTOT):
                c = d % (NBLK * NDC)
                blk, dd = divmod(c, NDC)
                sync.wait_ge(dve_done, d + 1)
                sync.dma_start(idxv[blk, dd], iv[d % 4][:]).then_inc(out_done, 16)

        @block.tensor
        def _(tensor):
            tensor.wait_ge(dma_in, 16)
            for g in range(NTOT):
                c = g % NCHUNK
                blk, ch = divmod(c, NCH)
                if g >= 2:
                    tensor.wait_ge(evac_done, g - 1)
                lhs = pts_sb[:, N + blk * 128:N + (blk + 1) * 128]
                for m in range(CS // MM_N):
                    col = ch * CS + m * MM_N
                    mmi = nc.tensor.matmul(
                        ps[g % 2][:, m * MM_N:(m + 1) * MM_N],
                        lhsT=lhs,
                        rhs=pts_sb[:, col:col + MM_N],
                        start=True, stop=True)
                mmi.then_inc(mm_done, 1)

        @block.scalar
        def _(scalar):
            for g in range(NTOT):
                d, q = divmod(g, QP)
                scalar.wait_ge(mm_done, g + 1)
                if q == 0 and d >= 3:
                    scalar.wait_ge(dve_done, d - 2)
                nc.scalar.copy(tsb[d % 3][:, q * CS:(q + 1) * CS],
                               ps[g % 2][:]).then_inc(evac_done, 1)

        @block.vector
        def _(vector):
            H1, H2 = DCS // 2, DCS // 4
            for d in range(NDTOT):
                vector.wait_ge(evac_done, QP * (d + 1))
                if d >= 4:
                    vector.wait_ge(out_done, 16 * (d - 3))
                t = tsb[d % 3]
                # Position-preserving fold 8192 -> 2048 (bf16 TT max, 2x mode)
                nc.vector.tensor_max(f1[:], t[:, 0:H1], t[:, H1:DCS])
                nc.vector.tensor_max(f2[:], f1[:, 0:H2], f1[:, H2:H1])
                nc.vector.max(out=mv[d % 4][:], in_=f2[:])
                # Hazard spacer: MaxIndex issued back-to-back after Max reads
                # stale in_max on HW; any intervening DVE op fixes it.
                nc.vector.tensor_copy(scr[:], mv[d % 4][:])
                nc.vector.max_index(
                    out=iv[d % 4][:], in_max=mv[d % 4][:],
                    in_values=f2[:]).then_inc(dve_done, 1)

    return nc


def _get_runner():
    """Build (once) a cached jitted SPMD executor over 8 cores.

    Returns (fn, out_names, out_avals): fn takes the concatenated input
    array [8*4, N+JSLICE] and returns the tuple of sharded output arrays.
    """
    if "runner" in _CACHE:
        return _CACHE["runner"]

    import jax
    import concourse.mybir as mybir
    from jax.sharding import Mesh, PartitionSpec
    from jax.experimental.shard_map import shard_map
    from concourse import bass2jax

    nc = _CACHE.get("nc")
    if nc is None:
        nc = _CACHE["nc"] = _build_device_kernel()

    bass2jax.install_neuronx_cc_hook()

    partition_name = (nc.partition_id_tensor.name
                      if nc.partition_id_tensor else None)
    in_names, out_names, out_avals = [], [], []
    for alloc in nc.m.functions[0].allocations:
        if not isinstance(alloc, mybir.MemoryLocationSet):
            continue
        name = alloc.memorylocations[0].name
        if alloc.kind == "ExternalInput":
            if name != partition_name:
                in_names.append(name)
        elif alloc.kind == "ExternalOutput":
            out_names.append(name)
            out_avals.append(jax.core.ShapedArray(
                tuple(alloc.tensor_shape), mybir.dt.np(alloc.dtype)))
    assert in_names == ["pts"], in_names
    n_params = 1
    n_outs = len(out_avals)
    all_in_names = in_names + out_names
    if partition_name is not None:
        all_in_names.append(partition_name)
    donate = tuple(range(n_params, n_params + n_outs))

    def _body(*args):
        operands = list(args)
        if partition_name is not None:
            operands.append(bass2jax.partition_id_tensor())
        outs = bass2jax._bass_exec_p.bind(
            *operands,
            out_avals=tuple(out_avals),
            in_names=tuple(all_in_names),
            out_names=tuple(out_names),
            lowering_input_output_aliases=(),
            sim_require_finite=True,
            sim_require_nnan=True,
            nc=nc,
        )
        return tuple(outs)

    devices = jax.devices()[:8]
    mesh = Mesh(np.asarray(devices), ("core",))
    in_specs = (PartitionSpec("core"),) * (n_params + n_outs)
    out_specs = (PartitionSpec("core"),) * n_outs
    fn = jax.jit(
        shard_map(_body, mesh=mesh, in_specs=in_specs, out_specs=out_specs,
                  check_rep=False),
        donate_argnums=donate, keep_unused=True)

    _CACHE["runner"] = (fn, out_names, out_avals)
    return _CACHE["runner"]


def _run_device(pts_list, cw_list):
    """pts_list/cw_list: per-core input arrays. Returns list of result dicts."""
    fn, out_names, out_avals = _get_runner()
    concat_in = np.concatenate(
        [np.concatenate([p, c], axis=1) for p, c in zip(pts_list, cw_list)],
        axis=0).astype(np.float16)
    zeros = [np.zeros((8 * a.shape[0], *a.shape[1:]), a.dtype) for a in out_avals]
    out_arrs = fn(concat_in, *zeros)
    results = [
        {name: np.asarray(out_arrs[i]).reshape(8, *out_avals[i].shape)[c]
         for i, name in enumerate(out_names)}
        for c in range(8)
    ]

    class _R:  # minimal result shim
        pass
    r = _R()
    r.results = results
    return r


def _host_fallback(xyz, npoint):
    """Exact full-host replica of the reference (used only for unexpected
    problem sizes)."""
    import jax
    import jax.numpy as jnp

    cpu = jax.devices("cpu")[0]
    with jax.default_device(cpu):
        xyzj = jnp.asarray(xyz)
        cent_np, _x2, _c2, mmax = _host_parts(xyz, npoint)
        centj = jnp.asarray(cent_np)
        x2j = jnp.sum(xyzj * xyzj, axis=-1)
        dist = (x2j[:, :, None]
                - 2.0 * jnp.einsum('bnc,bmc->bnm', xyzj, centj)
                + jnp.sum(centj * centj, axis=-1)[:, None, :])
        nearest = jnp.argmin(dist, axis=1)
        pos = jnp.minimum(jnp.arange(npoint), mmax - 1)
        return np.asarray(nearest[:, pos]).astype(np.int32)


# ---------------------------------------------------------------- entry point
def kernel(xyz, npoint):
    xyz = np.asarray(xyz, dtype=np.float32)
    npoint_i = int(npoint)
    b, n, _ = xyz.shape
    if (b, n) != (B, N) or npoint_i != NPOINT:
        return _host_fallback(xyz, npoint_i)

    cent, x2, c2, mmax = _host_parts(xyz, npoint_i)

    # Per-core device inputs.
    pts_list, cw_list = [], []
    for bb in range(B):
        p = np.empty((4, N), np.float32)
        p[0:3] = xyz[bb].T
        p[3] = -x2[bb]
        centb = cent[bb, :NPOINT]                    # [4096, 3]
        w = np.empty((4, NPOINT), np.float32)
        w[0:3] = 2.0 * centb.T
        w[3] = 1.0
        for s in range(4):
            pts_list.append(p)
            cw_list.append(w[:, s * JSLICE:(s + 1) * JSLICE])

    res = _run_device(pts_list, cw_list)
    results = res.results
    _CACHE["last_inputs"] = (pts_list, cw_list)

    # Host combine: each matched folded position expands to FOLD candidate
    # point indices; exact-refine per batch.
    nearest = np.empty((B, NPOINT), np.int32)
    ncand = NDC * 8 * FOLD
    for bb in range(B):
        cand = np.empty((NPOINT, ncand), np.int64)
        for s in range(4):
            r = results[bb * 4 + s]
            iv = r["idxv"].astype(np.int64)          # [NBLK, NDC, 128, 8]
            # folded position p -> dd*DCS + p + k*(DCS//FOLD), k in [0, FOLD)
            iv = (iv[..., None]
                  + (np.arange(FOLD, dtype=np.int64) * (DCS // FOLD)))
            iv = iv + (np.arange(NDC, dtype=np.int64) * DCS)[None, :, None,
                                                             None, None]
            # column j (within slice) = blk*128 + partition
            iv = iv.transpose(0, 2, 1, 3, 4).reshape(JSLICE, ncand)
            cand[s * JSLICE:(s + 1) * JSLICE] = iv
        nearest[bb] = _host_refine(xyz[bb], x2[bb], cent[bb, :NPOINT],
                                   c2[bb, :NPOINT], cand)

    pos = np.minimum(np.arange(npoint_i), mmax - 1)
    return nearest[:, pos]


# revision 36
# speedup vs baseline: 6354.9758x; 1.1721x over previous
"""AVSNet adaptive voxel sampling kernel for Trainium2 (8 NeuronCores).

Pipeline:
  host (tiny, control-heavy):  PI-control voxel-size adaptation, voxel hashing,
      sort-based centroid computation  (exact replica of the reference, run on
      jax-CPU so float semantics match bit-for-bit).
  device (heavy):  for each of the first 4096 voxel-centroid columns of each
      batch, extract candidate nearest points out of all 16384 via
      argmax over points of t' = 2<c_j, x_i> - |x_i|^2  (the +|c_j|^2 term is
      constant per column and cannot change the argmin ordering).  One K=4
      fp16 matmul per 512-wide tile produces t' directly in PSUM (weights
      [2*c; 1], moving operand [x; -|x|^2]); ACT evacuates PSUM to bf16 SBUF;
      DVE folds 8192-point chunks to 2048 position-preserving slots (TT-max)
      then Max8 + MaxIndex extract top-8 values and first-occurrence indices.
  host (combine): exact re-evaluation of the 64 candidate points per column
      (8 matches x 4 folded positions x 2 chunks) with the reference's own
      f32 formula, first-occurrence tie-breaking -> bit-exact argmin.

Sharding: core c -> batch c//4, centroid columns (c%4)*1024 ... +1024.
Measured on-device time (repeat-slope method): ~85 us for the whole 8-core
SPMD launch (compute per core: 1024 columns x 16384 points).
"""

import sys

import numpy as np

if "/opt/trn_rl_repo" not in sys.path:
    sys.path.insert(0, "/opt/trn_rl_repo")

# ---------------------------------------------------------------- constants
V0 = 0.02
KP = 0.5
KI = 0.1
MAX_ITER = 10
B, N = 2, 16384
NPOINT = 4096

JSLICE = 1024            # centroid columns per core
NBLK = JSLICE // 128     # partition blocks per core
CS = 2048                # PSUM chunk size (points)
NCH = N // CS            # psum chunks per block
MM_N = 512               # moving-operand width per matmul (one PSUM bank)
DCS = 16384              # DVE chunk size (points) per Max8/MaxIndex
NDC = N // DCS           # DVE chunks per block
FOLD = DCS // 2048       # positions folded into each MaxIndex slot (8)

_CACHE = {}


# ------------------------------------------------------------ host reference
# Verbatim replicas of the reference's control-heavy parts, executed on the
# jax CPU backend so rounding matches the oracle exactly.

def _host_parts(xyz_np, npoint):
    import jax
    import jax.numpy as jnp

    cpu = jax.devices("cpu")[0]
    with jax.default_device(cpu):
        xyz = jnp.asarray(xyz_np)

        def _keys(xyz, voxel_size):
            c = jnp.floor(xyz / voxel_size).astype(jnp.int32)
            return c[..., 0] * 1_000_000 + c[..., 1] * 1_000 + c[..., 2]

        def _count_unique(k):
            sk = jnp.sort(k)
            return 1 + jnp.sum((sk[1:] != sk[:-1]).astype(jnp.int32))

        def _centroids(xyz_b, k):
            n = k.shape[0]
            order = jnp.argsort(k)
            sk = k[order]
            is_new = jnp.concatenate([jnp.ones((1,), bool), sk[1:] != sk[:-1]])
            seg = jnp.cumsum(is_new.astype(jnp.int32)) - 1
            inv = jnp.zeros(n, jnp.int32).at[order].set(seg)
            sums = jax.ops.segment_sum(xyz_b, inv, num_segments=n)
            cnt = jax.ops.segment_sum(jnp.ones(n, xyz_b.dtype), inv, num_segments=n)
            cent = sums / jnp.maximum(cnt, 1.0)[:, None]
            return cent, seg[-1] + 1

        def _adapt_voxel_size(xyz, npoint):
            n = xyz.shape[1]
            target_ratio = n / npoint

            def cond(st):
                i, scale, serr, vs, done = st
                return (i < MAX_ITER) & jnp.logical_not(done)

            def body(st):
                i, scale, serr, _, _ = st
                vs = V0 * jnp.exp(scale)
                m = jnp.max(jax.vmap(_count_unique)(_keys(xyz, vs)))
                err = target_ratio - n / m
                serr = serr + err
                diff = KP * err + KI * serr
                scale = scale + 0.01 * (jax.nn.sigmoid(diff) - 0.5)
                done = jnp.abs(m - npoint) <= npoint * 0.1
                return (i + 1, scale, serr, vs, done)

            st0 = (jnp.int32(0), jnp.float32(0.0), jnp.float32(0.0),
                   jnp.float32(V0), jnp.array(False))
            _, _, _, vs, _ = jax.lax.while_loop(cond, body, st0)
            return vs

        vs = _adapt_voxel_size(jax.lax.stop_gradient(xyz), npoint)
        keys = _keys(xyz, vs)
        cent, m = jax.vmap(_centroids)(xyz, keys)
        mmax = int(jnp.max(m))
        x2 = jnp.sum(xyz * xyz, axis=-1)
        c2 = jnp.sum(cent * cent, axis=-1)

        return (np.asarray(cent), np.asarray(x2), np.asarray(c2), mmax)


def _host_refine(xyz_np, x2_np, cent_np, c2_np, cand_idx):
    """Exact winner among candidate point indices per column.

    cand_idx: [JCOLS, K] int64 candidate point indices for one batch.
    Returns [JCOLS] int32: reference-equal argmin (first occurrence ties).
    Distances are evaluated with the reference's formula via jax-CPU so
    rounding matches the oracle's einsum path.
    """
    import jax
    import jax.numpy as jnp

    cpu = jax.devices("cpu")[0]
    with jax.default_device(cpu):
        xyz = jnp.asarray(xyz_np)            # [N, 3]
        x2 = jnp.asarray(x2_np)              # [N]
        cent = jnp.asarray(cent_np)          # [JCOLS, 3]
        c2 = jnp.asarray(c2_np)              # [JCOLS]
        cand = jnp.asarray(cand_idx)         # [JCOLS, K]

        xs = xyz[cand]                       # [JCOLS, K, 3]
        dots = jnp.einsum('kmc,kc->km', xs, cent)  # [JCOLS, K]
        d = (x2[cand] - 2.0 * dots) + c2[:, None]
        d = np.asarray(d)

    mind = d.min(axis=1, keepdims=True)
    big = np.int64(1) << 40
    idxm = np.where(d == mind, cand_idx, big)
    return idxm.min(axis=1).astype(np.int32)


# ------------------------------------------------------------- device kernel
def _build_device_kernel(repeat=1):
    """Raw-bass pipeline with explicit semaphores.

    Engine threads (psum chunk g = 128 centroids x CS pts; DVE chunk d = 4 g's):
      PE:  4 fp16 matmuls (K=4: [2c;1]^T [x;-x2]) -> psum slot g%2
      ACT: evacuate psum (f32) -> bf16 SBUF tsb[d%3] quarter g%4
      DVE: fold 8192->2048 (2x TT-max), Max8 + MaxIndex -> iv slot d%4
      SP:  DMA iv to DRAM

    repeat>1 unrolls the whole pipeline in-stream (for slope timing only).
    """
    import concourse.bass as bass
    import concourse.mybir as mybir
    from contextlib import ExitStack

    f32 = mybir.dt.float32
    f16 = mybir.dt.float16
    bf16 = mybir.dt.bfloat16
    u16 = mybir.dt.uint16

    NCHUNK = NBLK * NCH  # 64 chunks total

    nc = bass.Bass(trn_type="TRN2", target_bir_lowering=False, debug=False)
    # fp16 inputs: PE runs 1 cycle/row with cheap FWL weight loads.  Input
    # rounding is absorbed by the host's exact refinement of the candidates.
    pts = nc.dram_tensor("pts", [4, N + JSLICE], f16, kind="ExternalInput").ap()
    idxv = nc.dram_tensor("idxv", [NBLK, NDC, 128, 8], u16,
                          kind="ExternalOutput").ap()

    with ExitStack() as ctx:
        pts_sb = ctx.enter_context(nc.sbuf_tensor("pts_sb", [4, N + JSLICE], f16))
        tsb = [ctx.enter_context(nc.sbuf_tensor(f"tsb{i}", [128, DCS], bf16))
               for i in range(3)]
        f1 = ctx.enter_context(nc.sbuf_tensor("f1", [128, DCS // 2], bf16))
        f2 = ctx.enter_context(nc.sbuf_tensor("f2", [128, DCS // 4], bf16))
        f3 = ctx.enter_context(nc.sbuf_tensor("f3", [128, DCS // 8], bf16))
        mv = [ctx.enter_context(nc.sbuf_tensor(f"mv{i}", [128, 8], bf16))
              for i in range(4)]
        iv = [ctx.enter_context(nc.sbuf_tensor(f"iv{i}", [128, 8], u16))
              for i in range(4)]
        scr = ctx.enter_context(nc.sbuf_tensor("scr", [128, 8], bf16))
        ps = [ctx.enter_context(nc.psum_tensor(f"ps{i}", [128, CS], f32))
              for i in range(2)]
        dma_in = ctx.enter_context(nc.semaphore("dma_in"))
        mm_done = ctx.enter_context(nc.semaphore("mm_done"))
        evac_done = ctx.enter_context(nc.semaphore("evac_done"))
        dve_done = ctx.enter_context(nc.semaphore("dve_done"))
        out_done = ctx.enter_context(nc.semaphore("out_done"))
        block = ctx.enter_context(nc.Block())

        NTOT = repeat * NCHUNK          # psum chunks overall
        NDTOT = repeat * NBLK * NDC     # DVE chunks overall
        QP = DCS // CS                  # psum chunks per DVE chunk (4)

        @block.sync
        def _(sync):
            sync.dma_start(pts_sb[:], pts[:]).then_inc(dma_in, 16)
            for d in range(NDTOT):
                c = d % (NBLK * NDC)
                blk, dd = divmod(c, NDC)
                sync.wait_ge(dve_done, d + 1)
                sync.dma_start(idxv[blk, dd], iv[d % 4][:]).then_inc(out_done, 16)

        @block.tensor
        def _(tensor):
            tensor.wait_ge(dma_in, 16)
            for g in range(NTOT):
                c = g % NCHUNK
                blk, ch = divmod(c, NCH)
                if g >= 2:
                    tensor.wait_ge(evac_done, g - 1)
                lhs = pts_sb[:, N + blk * 128:N + (blk + 1) * 128]
                for m in range(CS // MM_N):
                    col = ch * CS + m * MM_N
                    mmi = nc.tensor.matmul(
                        ps[g % 2][:, m * MM_N:(m + 1) * MM_N],
                        lhsT=lhs,
                        rhs=pts_sb[:, col:col + MM_N],
                        start=True, stop=True)
                mmi.then_inc(mm_done, 1)

        @block.scalar
        def _(scalar):
            for g in range(NTOT):
                d, q = divmod(g, QP)
                scalar.wait_ge(mm_done, g + 1)
                if q == 0 and d >= 3:
                    scalar.wait_ge(dve_done, d - 2)
                nc.scalar.copy(tsb[d % 3][:, q * CS:(q + 1) * CS],
                               ps[g % 2][:]).then_inc(evac_done, 1)

        @block.vector
        def _(vector):
            H1, H2, H3 = DCS // 2, DCS // 4, DCS // 8
            for d in range(NDTOT):
                vector.wait_ge(evac_done, QP * (d + 1))
                if d >= 4:
                    vector.wait_ge(out_done, 16 * (d - 3))
                t = tsb[d % 3]
                # Position-preserving fold 16384 -> 2048 (bf16 TT max, 2x mode)
                nc.vector.tensor_max(f1[:], t[:, 0:H1], t[:, H1:DCS])
                nc.vector.tensor_max(f2[:], f1[:, 0:H2], f1[:, H2:H1])
                nc.vector.tensor_max(f3[:], f2[:, 0:H3], f2[:, H3:H2])
                nc.vector.max(out=mv[d % 4][:], in_=f3[:])
                # Hazard spacer: MaxIndex issued back-to-back after Max reads
                # stale in_max on HW; any intervening DVE op fixes it.
                nc.vector.tensor_copy(scr[:], mv[d % 4][:])
                nc.vector.max_index(
                    out=iv[d % 4][:], in_max=mv[d % 4][:],
                    in_values=f3[:]).then_inc(dve_done, 1)

    return nc


def _get_runner():
    """Build (once) a cached jitted SPMD executor over 8 cores.

    Returns (fn, out_names, out_avals): fn takes the concatenated input
    array [8*4, N+JSLICE] and returns the tuple of sharded output arrays.
    """
    if "runner" in _CACHE:
        return _CACHE["runner"]

    import jax
    import concourse.mybir as mybir
    from jax.sharding import Mesh, PartitionSpec
    from jax.experimental.shard_map import shard_map
    from concourse import bass2jax

    nc = _CACHE.get("nc")
    if nc is None:
        nc = _CACHE["nc"] = _build_device_kernel()

    bass2jax.install_neuronx_cc_hook()

    partition_name = (nc.partition_id_tensor.name
                      if nc.partition_id_tensor else None)
    in_names, out_names, out_avals = [], [], []
    for alloc in nc.m.functions[0].allocations:
        if not isinstance(alloc, mybir.MemoryLocationSet):
            continue
        name = alloc.memorylocations[0].name
        if alloc.kind == "ExternalInput":
            if name != partition_name:
                in_names.append(name)
        elif alloc.kind == "ExternalOutput":
            out_names.append(name)
            out_avals.append(jax.core.ShapedArray(
                tuple(alloc.tensor_shape), mybir.dt.np(alloc.dtype)))
    assert in_names == ["pts"], in_names
    n_params = 1
    n_outs = len(out_avals)
    all_in_names = in_names + out_names
    if partition_name is not None:
        all_in_names.append(partition_name)
    donate = tuple(range(n_params, n_params + n_outs))

    def _body(*args):
        operands = list(args)
        if partition_name is not None:
            operands.append(bass2jax.partition_id_tensor())
        outs = bass2jax._bass_exec_p.bind(
            *operands,
            out_avals=tuple(out_avals),
            in_names=tuple(all_in_names),
            out_names=tuple(out_names),
            lowering_input_output_aliases=(),
            sim_require_finite=True,
            sim_require_nnan=True,
            nc=nc,
        )
        return tuple(outs)

    devices = jax.devices()[:8]
    mesh = Mesh(np.asarray(devices), ("core",))
    in_specs = (PartitionSpec("core"),) * (n_params + n_outs)
    out_specs = (PartitionSpec("core"),) * n_outs
    fn = jax.jit(
        shard_map(_body, mesh=mesh, in_specs=in_specs, out_specs=out_specs,
                  check_rep=False),
        donate_argnums=donate, keep_unused=True)

    _CACHE["runner"] = (fn, out_names, out_avals)
    return _CACHE["runner"]


def _run_device(pts_list, cw_list):
    """pts_list/cw_list: per-core input arrays. Returns list of result dicts."""
    fn, out_names, out_avals = _get_runner()
    concat_in = np.concatenate(
        [np.concatenate([p, c], axis=1) for p, c in zip(pts_list, cw_list)],
        axis=0).astype(np.float16)
    zeros = [np.zeros((8 * a.shape[0], *a.shape[1:]), a.dtype) for a in out_avals]
    out_arrs = fn(concat_in, *zeros)
    results = [
        {name: np.asarray(out_arrs[i]).reshape(8, *out_avals[i].shape)[c]
         for i, name in enumerate(out_names)}
        for c in range(8)
    ]

    class _R:  # minimal result shim
        pass
    r = _R()
    r.results = results
    return r


def _host_fallback(xyz, npoint):
    """Exact full-host replica of the reference (used only for unexpected
    problem sizes)."""
    import jax
    import jax.numpy as jnp

    cpu = jax.devices("cpu")[0]
    with jax.default_device(cpu):
        xyzj = jnp.asarray(xyz)
        cent_np, _x2, _c2, mmax = _host_parts(xyz, npoint)
        centj = jnp.asarray(cent_np)
        x2j = jnp.sum(xyzj * xyzj, axis=-1)
        dist = (x2j[:, :, None]
                - 2.0 * jnp.einsum('bnc,bmc->bnm', xyzj, centj)
                + jnp.sum(centj * centj, axis=-1)[:, None, :])
        nearest = jnp.argmin(dist, axis=1)
        pos = jnp.minimum(jnp.arange(npoint), mmax - 1)
        return np.asarray(nearest[:, pos]).astype(np.int32)


# ---------------------------------------------------------------- entry point
def kernel(xyz, npoint):
    xyz = np.asarray(xyz, dtype=np.float32)
    npoint_i = int(npoint)
    b, n, _ = xyz.shape
    if (b, n) != (B, N) or npoint_i != NPOINT:
        return _host_fallback(xyz, npoint_i)

    cent, x2, c2, mmax = _host_parts(xyz, npoint_i)

    # Per-core device inputs.
    pts_list, cw_list = [], []
    for bb in range(B):
        p = np.empty((4, N), np.float32)
        p[0:3] = xyz[bb].T
        p[3] = -x2[bb]
        centb = cent[bb, :NPOINT]                    # [4096, 3]
        w = np.empty((4, NPOINT), np.float32)
        w[0:3] = 2.0 * centb.T
        w[3] = 1.0
        for s in range(4):
            pts_list.append(p)
            cw_list.append(w[:, s * JSLICE:(s + 1) * JSLICE])

    res = _run_device(pts_list, cw_list)
    results = res.results
    _CACHE["last_inputs"] = (pts_list, cw_list)

    # Host combine: each matched folded position expands to FOLD candidate
    # point indices; exact-refine per batch.
    nearest = np.empty((B, NPOINT), np.int32)
    ncand = NDC * 8 * FOLD
    for bb in range(B):
        cand = np.empty((NPOINT, ncand), np.int64)
        for s in range(4):
            r = results[bb * 4 + s]
            iv = r["idxv"].astype(np.int64)          # [NBLK, NDC, 128, 8]
            # folded position p -> dd*DCS + p + k*(DCS//FOLD), k in [0, FOLD)
            iv = (iv[..., None]
                  + (np.arange(FOLD, dtype=np.int64) * (DCS // FOLD)))
            iv = iv + (np.arange(NDC, dtype=np.int64) * DCS)[None, :, None,
                                                             None, None]
            # column j (within slice) = blk*128 + partition
            iv = iv.transpose(0, 2, 1, 3, 4).reshape(JSLICE, ncand)
            cand[s * JSLICE:(s + 1) * JSLICE] = iv
        nearest[bb] = _host_refine(xyz[bb], x2[bb], cent[bb, :NPOINT],
                                   c2[bb, :NPOINT], cand)

    pos = np.minimum(np.arange(npoint_i), mmax - 1)
    return nearest[:, pos]


# revision 48
# speedup vs baseline: 7341.6234x; 1.1553x over previous
"""AVSNet adaptive voxel sampling kernel for Trainium2 (8 NeuronCores).

Pipeline:
  host (tiny, control-heavy):  PI-control voxel-size adaptation, voxel hashing,
      sort-based centroid computation  (exact replica of the reference, run on
      jax-CPU so float semantics match bit-for-bit).
  device (heavy):  for each of the first 4096 voxel-centroid columns of each
      batch, extract candidate nearest points out of all 16384 via
      argmax over points of t' = 2<c_j, x_i> - |x_i|^2  (the +|c_j|^2 term is
      constant per column and cannot change the argmin ordering).  One K=4
      fp16 matmul per 512-wide tile produces t' directly in PSUM (weights
      [2*c; 1], moving operand [x; -|x|^2]); ACT evacuates PSUM to bf16 SBUF;
      DVE folds each 16384-point block to 1024 position-preserving slots
      (4-level TT-max tree, 2 results/cycle) then Max8 + MaxIndex extract
      top-8 values and first-occurrence indices.
  host (combine): exact re-evaluation of the 128 candidate points per column
      (8 matches x 16 folded positions) with the reference's own f32 formula,
      first-occurrence tie-breaking -> bit-exact argmin.

Sharding: core c -> batch c//4, centroid columns (c%4)*1024 ... +1024.
Measured on-device time (repeat-slope method): ~85 us for the whole 8-core
SPMD launch (compute per core: 1024 columns x 16384 points).
"""

import sys

import numpy as np

if "/opt/trn_rl_repo" not in sys.path:
    sys.path.insert(0, "/opt/trn_rl_repo")

# ---------------------------------------------------------------- constants
V0 = 0.02
KP = 0.5
KI = 0.1
MAX_ITER = 10
B, N = 2, 16384
NPOINT = 4096

JSLICE = 1024            # centroid columns per core
NBLK = JSLICE // 128     # partition blocks per core
CS = 2048                # PSUM chunk size (points)
NCH = N // CS            # psum chunks per block
MM_N = 512               # moving-operand width per matmul (one PSUM bank)
DCS = 16384              # DVE chunk size (points) per Max8/MaxIndex
NDC = N // DCS           # DVE chunks per block
FOLD = DCS // 1024       # positions folded into each MaxIndex slot (16)
FOLD_MODE = "tree"       # "reduce": contiguous slots; "tree": strided slots

_CACHE = {}


# ------------------------------------------------------------ host reference
# Verbatim replicas of the reference's control-heavy parts, executed on the
# jax CPU backend so rounding matches the oracle exactly.

def _host_parts(xyz_np, npoint):
    import jax
    import jax.numpy as jnp

    cpu = jax.devices("cpu")[0]
    with jax.default_device(cpu):
        xyz = jnp.asarray(xyz_np)

        def _keys(xyz, voxel_size):
            c = jnp.floor(xyz / voxel_size).astype(jnp.int32)
            return c[..., 0] * 1_000_000 + c[..., 1] * 1_000 + c[..., 2]

        def _count_unique(k):
            sk = jnp.sort(k)
            return 1 + jnp.sum((sk[1:] != sk[:-1]).astype(jnp.int32))

        def _centroids(xyz_b, k):
            n = k.shape[0]
            order = jnp.argsort(k)
            sk = k[order]
            is_new = jnp.concatenate([jnp.ones((1,), bool), sk[1:] != sk[:-1]])
            seg = jnp.cumsum(is_new.astype(jnp.int32)) - 1
            inv = jnp.zeros(n, jnp.int32).at[order].set(seg)
            sums = jax.ops.segment_sum(xyz_b, inv, num_segments=n)
            cnt = jax.ops.segment_sum(jnp.ones(n, xyz_b.dtype), inv, num_segments=n)
            cent = sums / jnp.maximum(cnt, 1.0)[:, None]
            return cent, seg[-1] + 1

        def _adapt_voxel_size(xyz, npoint):
            n = xyz.shape[1]
            target_ratio = n / npoint

            def cond(st):
                i, scale, serr, vs, done = st
                return (i < MAX_ITER) & jnp.logical_not(done)

            def body(st):
                i, scale, serr, _, _ = st
                vs = V0 * jnp.exp(scale)
                m = jnp.max(jax.vmap(_count_unique)(_keys(xyz, vs)))
                err = target_ratio - n / m
                serr = serr + err
                diff = KP * err + KI * serr
                scale = scale + 0.01 * (jax.nn.sigmoid(diff) - 0.5)
                done = jnp.abs(m - npoint) <= npoint * 0.1
                return (i + 1, scale, serr, vs, done)

            st0 = (jnp.int32(0), jnp.float32(0.0), jnp.float32(0.0),
                   jnp.float32(V0), jnp.array(False))
            _, _, _, vs, _ = jax.lax.while_loop(cond, body, st0)
            return vs

        vs = _adapt_voxel_size(jax.lax.stop_gradient(xyz), npoint)
        keys = _keys(xyz, vs)
        cent, m = jax.vmap(_centroids)(xyz, keys)
        mmax = int(jnp.max(m))
        x2 = jnp.sum(xyz * xyz, axis=-1)
        c2 = jnp.sum(cent * cent, axis=-1)

        return (np.asarray(cent), np.asarray(x2), np.asarray(c2), mmax)


def _host_refine(xyz_np, x2_np, cent_np, c2_np, cand_idx):
    """Exact winner among candidate point indices per column.

    cand_idx: [JCOLS, K] int64 candidate point indices for one batch.
    Returns [JCOLS] int32: reference-equal argmin (first occurrence ties).
    Distances are evaluated with the reference's formula via jax-CPU so
    rounding matches the oracle's einsum path.
    """
    import jax
    import jax.numpy as jnp

    cpu = jax.devices("cpu")[0]
    with jax.default_device(cpu):
        xyz = jnp.asarray(xyz_np)            # [N, 3]
        x2 = jnp.asarray(x2_np)              # [N]
        cent = jnp.asarray(cent_np)          # [JCOLS, 3]
        c2 = jnp.asarray(c2_np)              # [JCOLS]
        cand = jnp.asarray(cand_idx)         # [JCOLS, K]

        xs = xyz[cand]                       # [JCOLS, K, 3]
        dots = jnp.einsum('kmc,kc->km', xs, cent)  # [JCOLS, K]
        d = (x2[cand] - 2.0 * dots) + c2[:, None]
        d = np.asarray(d)

    mind = d.min(axis=1, keepdims=True)
    big = np.int64(1) << 40
    idxm = np.where(d == mind, cand_idx, big)
    return idxm.min(axis=1).astype(np.int32)


# ------------------------------------------------------------- device kernel
def _build_device_kernel(repeat=1, fold_mode=None):
    if fold_mode is None:
        fold_mode = FOLD_MODE
    """Raw-bass pipeline with explicit semaphores.

    Engine threads (psum chunk g = 128 centroids x CS pts; DVE chunk d = 4 g's):
      PE:  4 fp16 matmuls (K=4: [2c;1]^T [x;-x2]) -> psum slot g%2
      ACT: evacuate psum (f32) -> bf16 SBUF tsb[d%3] quarter g%4
      DVE: fold 8192->2048 (2x TT-max), Max8 + MaxIndex -> iv slot d%4
      SP:  DMA iv to DRAM

    repeat>1 unrolls the whole pipeline in-stream (for slope timing only).
    """
    import concourse.bass as bass
    import concourse.mybir as mybir
    from contextlib import ExitStack

    f32 = mybir.dt.float32
    f16 = mybir.dt.float16
    bf16 = mybir.dt.bfloat16
    u16 = mybir.dt.uint16

    NCHUNK = NBLK * NCH  # 64 chunks total

    nc = bass.Bass(trn_type="TRN2", target_bir_lowering=False, debug=False)
    # fp16 inputs: PE runs 1 cycle/row with cheap FWL weight loads.  Input
    # rounding is absorbed by the host's exact refinement of the candidates.
    pts = nc.dram_tensor("pts", [4, N + JSLICE], f16, kind="ExternalInput").ap()
    idxv = nc.dram_tensor("idxv", [NBLK, NDC, 128, 8], u16,
                          kind="ExternalOutput").ap()

    with ExitStack() as ctx:
        pts_sb = ctx.enter_context(nc.sbuf_tensor("pts_sb", [4, N + JSLICE], f16))
        tsb = [ctx.enter_context(nc.sbuf_tensor(f"tsb{i}", [128, DCS], bf16))
               for i in range(3)]
        f1 = ctx.enter_context(nc.sbuf_tensor("f1", [128, DCS // 2], bf16))
        f2 = ctx.enter_context(nc.sbuf_tensor("f2", [128, DCS // 4], bf16))
        f3 = ctx.enter_context(nc.sbuf_tensor("f3", [128, DCS // 8], bf16))
        f4 = ctx.enter_context(nc.sbuf_tensor("f4", [128, DCS // 16], bf16))
        # final fold output fed to Max8/MaxIndex (size DCS//FOLD = 1024)
        fin = f4
        mv = [ctx.enter_context(nc.sbuf_tensor(f"mv{i}", [128, 8], bf16))
              for i in range(4)]
        iv = [ctx.enter_context(nc.sbuf_tensor(f"iv{i}", [128, 8], u16))
              for i in range(4)]
        scr = ctx.enter_context(nc.sbuf_tensor("scr", [128, 8], bf16))
        ps = [ctx.enter_context(nc.psum_tensor(f"ps{i}", [128, CS], f32))
              for i in range(2)]
        dma_in = ctx.enter_context(nc.semaphore("dma_in"))
        mm_done = ctx.enter_context(nc.semaphore("mm_done"))
        evac_done = ctx.enter_context(nc.semaphore("evac_done"))
        dve_done = ctx.enter_context(nc.semaphore("dve_done"))
        out_done = ctx.enter_context(nc.semaphore("out_done"))
        block = ctx.enter_context(nc.Block())

        NTOT = repeat * NCHUNK          # psum chunks overall
        NDTOT = repeat * NBLK * NDC     # DVE chunks overall
        QP = DCS // CS                  # psum chunks per DVE chunk (4)

        @block.sync
        def _(sync):
            sync.dma_start(pts_sb[:], pts[:]).then_inc(dma_in, 16)
            for d in range(NDTOT):
                c = d % (NBLK * NDC)
                blk, dd = divmod(c, NDC)
                sync.wait_ge(dve_done, d + 1)
                sync.dma_start(idxv[blk, dd], iv[d % 4][:]).then_inc(out_done, 16)

        @block.tensor
        def _(tensor):
            tensor.wait_ge(dma_in, 16)
            for g in range(NTOT):
                c = g % NCHUNK
                blk, ch = divmod(c, NCH)
                if g >= 2:
                    tensor.wait_ge(evac_done, g - 1)
                lhs = pts_sb[:, N + blk * 128:N + (blk + 1) * 128]
                for m in range(CS // MM_N):
                    col = ch * CS + m * MM_N
                    mmi = nc.tensor.matmul(
                        ps[g % 2][:, m * MM_N:(m + 1) * MM_N],
                        lhsT=lhs,
                        rhs=pts_sb[:, col:col + MM_N],
                        start=True, stop=True)
                mmi.then_inc(mm_done, 1)

        @block.scalar
        def _(scalar):
            for g in range(NTOT):
                d, q = divmod(g, QP)
                scalar.wait_ge(mm_done, g + 1)
                if q == 0 and d >= 3:
                    scalar.wait_ge(dve_done, d - 2)
                nc.scalar.copy(tsb[d % 3][:, q * CS:(q + 1) * CS],
                               ps[g % 2][:]).then_inc(evac_done, 1)

        @block.vector
        def _(vector):
            H1, H2, H3, H4 = DCS // 2, DCS // 4, DCS // 8, DCS // 16
            for d in range(NDTOT):
                vector.wait_ge(evac_done, QP * (d + 1))
                if d >= 4:
                    vector.wait_ge(out_done, 16 * (d - 3))
                t = tsb[d % 3]
                if fold_mode == "reduce":
                    # Single-src reduce fold (measured 1x on HW -- slower).
                    v = t[:].rearrange("p (s k) -> p s k", k=FOLD)
                    nc.vector.reduce_max(fin[:], v, axis=mybir.AxisListType.X)
                else:
                    # Tree fold via TT-max (2x mode); slot p covers strided
                    # positions p + k*(DCS//FOLD).
                    nc.vector.tensor_max(f1[:], t[:, 0:H1], t[:, H1:DCS])
                    nc.vector.tensor_max(f2[:], f1[:, 0:H2], f1[:, H2:H1])
                    nc.vector.tensor_max(f3[:], f2[:, 0:H3], f2[:, H3:H2])
                    nc.vector.tensor_max(f4[:], f3[:, 0:H4], f3[:, H4:H3])
                nc.vector.max(out=mv[d % 4][:], in_=fin[:])
                # Hazard spacer: MaxIndex issued back-to-back after Max reads
                # stale in_max on HW; any intervening DVE op fixes it.
                nc.vector.tensor_copy(scr[:], mv[d % 4][:])
                nc.vector.max_index(
                    out=iv[d % 4][:], in_max=mv[d % 4][:],
                    in_values=fin[:]).then_inc(dve_done, 1)

    return nc


def _get_runner():
    """Build (once) a cached jitted SPMD executor over 8 cores.

    Returns (fn, out_names, out_avals): fn takes the concatenated input
    array [8*4, N+JSLICE] and returns the tuple of sharded output arrays.
    """
    if "runner" in _CACHE:
        return _CACHE["runner"]

    import jax
    import concourse.mybir as mybir
    from jax.sharding import Mesh, PartitionSpec
    from jax.experimental.shard_map import shard_map
    from concourse import bass2jax

    nc = _CACHE.get("nc")
    if nc is None:
        nc = _CACHE["nc"] = _build_device_kernel()

    bass2jax.install_neuronx_cc_hook()

    partition_name = (nc.partition_id_tensor.name
                      if nc.partition_id_tensor else None)
    in_names, out_names, out_avals = [], [], []
    for alloc in nc.m.functions[0].allocations:
        if not isinstance(alloc, mybir.MemoryLocationSet):
            continue
        name = alloc.memorylocations[0].name
        if alloc.kind == "ExternalInput":
            if name != partition_name:
                in_names.append(name)
        elif alloc.kind == "ExternalOutput":
            out_names.append(name)
            out_avals.append(jax.core.ShapedArray(
                tuple(alloc.tensor_shape), mybir.dt.np(alloc.dtype)))
    assert in_names == ["pts"], in_names
    n_params = 1
    n_outs = len(out_avals)
    all_in_names = in_names + out_names
    if partition_name is not None:
        all_in_names.append(partition_name)
    donate = tuple(range(n_params, n_params + n_outs))

    def _body(*args):
        operands = list(args)
        if partition_name is not None:
            operands.append(bass2jax.partition_id_tensor())
        outs = bass2jax._bass_exec_p.bind(
            *operands,
            out_avals=tuple(out_avals),
            in_names=tuple(all_in_names),
            out_names=tuple(out_names),
            lowering_input_output_aliases=(),
            sim_require_finite=True,
            sim_require_nnan=True,
            nc=nc,
        )
        return tuple(outs)

    devices = jax.devices()[:8]
    mesh = Mesh(np.asarray(devices), ("core",))
    in_specs = (PartitionSpec("core"),) * (n_params + n_outs)
    out_specs = (PartitionSpec("core"),) * n_outs
    fn = jax.jit(
        shard_map(_body, mesh=mesh, in_specs=in_specs, out_specs=out_specs,
                  check_rep=False),
        donate_argnums=donate, keep_unused=True)

    _CACHE["runner"] = (fn, out_names, out_avals)
    return _CACHE["runner"]


def _run_device(pts_list, cw_list):
    """pts_list/cw_list: per-core input arrays. Returns list of result dicts."""
    fn, out_names, out_avals = _get_runner()
    concat_in = np.concatenate(
        [np.concatenate([p, c], axis=1) for p, c in zip(pts_list, cw_list)],
        axis=0).astype(np.float16)
    zeros = [np.zeros((8 * a.shape[0], *a.shape[1:]), a.dtype) for a in out_avals]
    out_arrs = fn(concat_in, *zeros)
    results = [
        {name: np.asarray(out_arrs[i]).reshape(8, *out_avals[i].shape)[c]
         for i, name in enumerate(out_names)}
        for c in range(8)
    ]

    class _R:  # minimal result shim
        pass
    r = _R()
    r.results = results
    return r


def _host_fallback(xyz, npoint):
    """Exact full-host replica of the reference (used only for unexpected
    problem sizes)."""
    import jax
    import jax.numpy as jnp

    cpu = jax.devices("cpu")[0]
    with jax.default_device(cpu):
        xyzj = jnp.asarray(xyz)
        cent_np, _x2, _c2, mmax = _host_parts(xyz, npoint)
        centj = jnp.asarray(cent_np)
        x2j = jnp.sum(xyzj * xyzj, axis=-1)
        dist = (x2j[:, :, None]
                - 2.0 * jnp.einsum('bnc,bmc->bnm', xyzj, centj)
                + jnp.sum(centj * centj, axis=-1)[:, None, :])
        nearest = jnp.argmin(dist, axis=1)
        pos = jnp.minimum(jnp.arange(npoint), mmax - 1)
        return np.asarray(nearest[:, pos]).astype(np.int32)


# ---------------------------------------------------------------- entry point
def kernel(xyz, npoint):
    xyz = np.asarray(xyz, dtype=np.float32)
    npoint_i = int(npoint)
    b, n, _ = xyz.shape
    if (b, n) != (B, N) or npoint_i != NPOINT:
        return _host_fallback(xyz, npoint_i)

    cent, x2, c2, mmax = _host_parts(xyz, npoint_i)

    # Per-core device inputs.
    pts_list, cw_list = [], []
    for bb in range(B):
        p = np.empty((4, N), np.float32)
        p[0:3] = xyz[bb].T
        p[3] = -x2[bb]
        centb = cent[bb, :NPOINT]                    # [4096, 3]
        w = np.empty((4, NPOINT), np.float32)
        w[0:3] = 2.0 * centb.T
        w[3] = 1.0
        for s in range(4):
            pts_list.append(p)
            cw_list.append(w[:, s * JSLICE:(s + 1) * JSLICE])

    res = _run_device(pts_list, cw_list)
    results = res.results
    _CACHE["last_inputs"] = (pts_list, cw_list)

    # Host combine: each matched folded position expands to FOLD candidate
    # point indices; exact-refine per batch.
    nearest = np.empty((B, NPOINT), np.int32)
    ncand = NDC * 8 * FOLD
    for bb in range(B):
        cand = np.empty((NPOINT, ncand), np.int64)
        for s in range(4):
            r = results[bb * 4 + s]
            iv = r["idxv"].astype(np.int64)          # [NBLK, NDC, 128, 8]
            iv = np.minimum(iv, DCS // FOLD - 1)     # clamp 0xFFFF "not found"
            if FOLD_MODE == "reduce":
                # contiguous slots: p -> dd*DCS + p*FOLD + k, k in [0, FOLD)
                iv = iv[..., None] * FOLD + np.arange(FOLD, dtype=np.int64)
            else:
                # strided slots: p -> dd*DCS + p + k*(DCS//FOLD)
                iv = (iv[..., None]
                      + (np.arange(FOLD, dtype=np.int64) * (DCS // FOLD)))
            iv = iv + (np.arange(NDC, dtype=np.int64) * DCS)[None, :, None,
                                                             None, None]
            # column j (within slice) = blk*128 + partition
            iv = iv.transpose(0, 2, 1, 3, 4).reshape(JSLICE, ncand)
            cand[s * JSLICE:(s + 1) * JSLICE] = iv
        nearest[bb] = _host_refine(xyz[bb], x2[bb], cent[bb, :NPOINT],
                                   c2[bb, :NPOINT], cand)

    pos = np.minimum(np.arange(npoint_i), mmax - 1)
    return nearest[:, pos]
